# revision 28
# baseline (speedup 1.0000x reference)
"""ChunkMHSA (banded local-window attention) Trainium2 kernel.

Full-input contract: kernel(**inputs) takes the complete tensors from
setup_inputs() and returns the full [B, T, D] output.  Internally the
sequence dimension is sharded 8 ways (256 queries per NeuronCore) with a
front/back halo of 6/3 tokens, so each core runs the whole fused pipeline
(LayerNorm -> QKV -> banded softmax(QK^T)V -> output projection ->
residual) independently -- no collectives.

Per-core dataflow (SPMD, one Bass program):
  x[tok,D] f32 --bn_stats--> mean/rstd --ts--> xr f16 --PE transpose-->
  xTr[D,tok] --PE f16 matmuls--> q,k [hk,tok] and vT [tok,hk]
  scores psum[q,s] = mask + q.k ; ACT exp(accum sums) ; normalize ;
  PE transpose -> attnT[s,q] ; ctx[q,hk] ; PE transpose -> ctxT[hk,q] ;
  out psum[q,D] = ctxT.Wo + I.x (residual) ; DMA psum -> DRAM.
"""

import os

os.environ.setdefault("JAX_PLATFORMS", "axon")

from contextlib import ExitStack

import numpy as np

import concourse.bass as bass
import concourse.bacc as bacc
import concourse.tile as tile
from concourse import mybir
from concourse.bass_utils import run_bass_kernel_spmd

F32 = mybir.dt.float32
F32R = mybir.dt.float32r
F16 = mybir.dt.float16

B, T, D = 2, 2048, 512
H, DH = 8, 64
WF, WB = 6, 3
LN_EPS = 1e-3
NCORES = 8
TLOC = T // NCORES          # 256 queries per core
TIN = WF + TLOC + WB        # 265 local tokens incl. halo
NTT = 3                     # token tiles per batch (128+128+9)
NQC = 2                     # query chunks of 128 per batch
S = 128 + WF + WB           # 137 keys per query chunk
NEG = -30000.0              # additive mask value (fp16-safe)

_CACHE = {}


def _build_program():
    nc = bacc.Bacc(
        "TRN2", target_bir_lowering=False, debug=False, num_devices=NCORES
    )

    xs = nc.dram_tensor("xs", [B, TIN, D], F32, kind="ExternalInput").ap()
    wq = nc.dram_tensor("wq", [4, 128, D], F16, kind="ExternalInput").ap()
    wk = nc.dram_tensor("wk", [4, 128, D], F16, kind="ExternalInput").ap()
    wv = nc.dram_tensor("wv", [4, 128, D], F16, kind="ExternalInput").ap()
    wo = nc.dram_tensor("wo", [4, 128, D], F16, kind="ExternalInput").ap()
    maskd = nc.dram_tensor("maskd", [NQC, 128, S], F16, kind="ExternalInput").ap()
    eye16d = nc.dram_tensor("eye16", [128, 128], F16, kind="ExternalInput").ap()
    xq32d = nc.dram_tensor("xq32", [B, NQC, 128, D], F32, kind="ExternalInput").ap()
    outd = nc.dram_tensor("out", [B, TLOC, D], F32, kind="ExternalOutput").ap()

    with tile.TileContext(nc) as tc, ExitStack() as ctx:
        _emit(ctx, tc, xs, wq, wk, wv, wo, maskd, eye16d, xq32d, outd)

    nc.compile()
    return nc


def _emit(ctx, tc, xs, wq, wk, wv, wo, maskd, eye16d, xq32d, outd):
    nc = tc.nc
    EXP = mybir.ActivationFunctionType.Exp
    SQRT = mybir.ActivationFunctionType.Sqrt
    COPY = mybir.ActivationFunctionType.Copy
    SUB = mybir.AluOpType.subtract
    MULT = mybir.AluOpType.mult

    consts = ctx.enter_context(tc.tile_pool(name="consts", bufs=1))
    persist = ctx.enter_context(tc.tile_pool(name="persist", bufs=1))
    ln_tmp = ctx.enter_context(tc.tile_pool(name="ln_tmp", bufs=3))
    xr_pool = ctx.enter_context(tc.tile_pool(name="xr", bufs=3))
    attn_tmp = ctx.enter_context(tc.tile_pool(name="attn_tmp", bufs=4))
    # PSUM budget is 8 banks; tags: tr(1) pqk(1) pv(1) sc(2) atm(1) att(1)
    # ctx(1) = 8.  Output-projection psum reuses the "pv" tag/slot.
    ps_tr = ctx.enter_context(tc.tile_pool(name="ps_tr", bufs=1, space="PSUM"))
    ps_proj = ctx.enter_context(tc.tile_pool(name="ps_proj", bufs=1, space="PSUM"))
    ps_sc = ctx.enter_context(tc.tile_pool(name="ps_sc", bufs=2, space="PSUM"))
    ps_at = ctx.enter_context(tc.tile_pool(name="ps_at", bufs=1, space="PSUM"))
    ps_ctx = ctx.enter_context(tc.tile_pool(name="ps_ctx", bufs=1, space="PSUM"))

    # ---- constants / weights -------------------------------------------------
    eye16 = consts.tile([128, 128], F16)
    nc.sync.dma_start(eye16, eye16d)
    xq32 = consts.tile([128, B * NQC, D], F32)
    nc.sync.dma_start(xq32, xq32d.rearrange("b c p d -> p (b c) d"))
    mask_sb = consts.tile([128, NQC, S], F16)
    nc.sync.dma_start(mask_sb, maskd.rearrange("c p s -> p c s"))
    epst = consts.tile([128, 1], F32)
    nc.vector.memset(epst, LN_EPS)

    w_sb = {}
    for name, wd in (("q", wq), ("k", wk), ("v", wv), ("o", wo)):
        w = consts.tile([128, 4, D], F16, tag=f"w{name}")
        nc.sync.dma_start(w, wd.rearrange("j p d -> p j d"))
        w_sb[name] = w

    # ---- x load + LayerNorm stats + centered/scaled xr + transpose ----------
    # x_sb: 6 token tiles (batch b tile i at [:, b*3+i, :]); local token
    # l of batch b lives at partition l%128 of tile b*3 + l//128.
    x_sb = persist.tile([128, 2 * NTT, D], F32)
    xtr = persist.tile([128, 4, 2 * 384], F16)   # [dpart, dchunk, b*384+tok]
    q_sb = persist.tile([128, 4, B, TIN], F16, tag="q_sb")
    k_sb = persist.tile([128, 4, B, TIN], F16, tag="k_sb")
    vt_sb = persist.tile([128, B, NTT, D], F16, tag="vt_sb")

    for b in range(B):
        nc.gpsimd.memset(x_sb[:, b * NTT + 2, :], 0.0)
    for b in range(B):
        for i in range(NTT):
            rows = min(128, TIN - 128 * i)
            nc.sync.dma_start(
                x_sb[:rows, b * NTT + i, :], xs[b, 128 * i : 128 * i + rows, :]
            )

    for b in range(B):
        for i in range(NTT):
            xt = x_sb[:, b * NTT + i, :]
            st = ln_tmp.tile([128, 6], F32, tag="st")
            mv = ln_tmp.tile([128, 2], F32, tag="mv")
            nc.vector.bn_stats(out=st, in_=xt)
            nc.vector.bn_aggr(out=mv, in_=st)
            sd = ln_tmp.tile([128, 1], F32, tag="sd")
            nc.scalar.activation(out=sd, in_=mv[:, 1:2], func=SQRT, bias=epst)
            rstd = ln_tmp.tile([128, 1], F32, tag="rstd")
            nc.vector.reciprocal(out=rstd, in_=sd)
            xr = xr_pool.tile([128, D], F16, tag="xr")
            nc.vector.tensor_scalar(
                out=xr, in0=xt, scalar1=mv[:, 0:1], scalar2=rstd,
                op0=SUB, op1=MULT,
            )
            # transpose 4 d-chunks of xr into psum, then evac to xtr
            pt = ps_tr.tile([128, 4, 128], F16, tag="tr")
            for j in range(4):
                nc.tensor.transpose(pt[:, j, :], xr[:, 128 * j : 128 * j + 128], eye16)
            for j in range(4):
                nc.vector.tensor_copy(xtr[:, j, 384 * b + 128 * i : 384 * b + 128 * (i + 1)], pt[:, j, :])

    # ---- projections ---------------------------------------------------------
    # q,k: [hk, tok] = W.T @ xTr ; vT: [tok, hk] = xTr.T @ Wv
    for b in range(B):
        for name, dst in (("q", q_sb), ("k", k_sb)):
            for hkt in range(4):
                ps = ps_proj.tile([128, TIN], F32, tag="pqk")
                for j in range(4):
                    nc.tensor.matmul(
                        ps,
                        w_sb[name][:, j, 128 * hkt : 128 * (hkt + 1)],
                        xtr[:, j, 384 * b : 384 * b + TIN],
                        start=(j == 0), stop=(j == 3),
                    )
                nc.scalar.activation(out=dst[:, hkt, b, :], in_=ps, func=COPY)
        for i in range(NTT):
            ps = ps_proj.tile([128, D], F32, tag="pv")
            for j in range(4):
                nc.tensor.matmul(
                    ps,
                    xtr[:, j, 384 * b + 128 * i : 384 * b + 128 * (i + 1)],
                    w_sb["v"][:, j, :],
                    start=(j == 0), stop=(j == 3),
                )
            nc.scalar.activation(out=vt_sb[:, b, i, :], in_=ps, func=COPY)

    # ---- attention + output projection --------------------------------------
    for b in range(B):
        for cq in range(NQC):
            q0 = WF + 128 * cq          # local token of first query
            s0 = 128 * cq               # local token of first key
            at_m = ps_at.tile([128, 8, 128], F16, tag="atm")   # attnT main
            at_t = ps_at.tile([9, 8, 128], F16, tag="att")     # attnT tail
            ctxp = ps_ctx.tile([128, 8, DH], F32, tag="ctx")
            for h in range(8):
                hp = 64 * (h % 2)
                hkt = h // 2
                sc = ps_sc.tile([128, S], F32, tag="sc")
                nc.tensor.matmul(sc, eye16, mask_sb[:, cq, :], start=True, stop=False)
                nc.tensor.matmul(
                    sc,
                    q_sb[hp : hp + 64, hkt, b, q0 : q0 + 128],
                    k_sb[hp : hp + 64, hkt, b, s0 : s0 + S],
                    start=False, stop=True,
                )
                ea = attn_tmp.tile([128, S], F16, tag="ea")
                sums = attn_tmp.tile([128, 1], F32, tag="sums")
                nc.scalar.activation(
                    out=ea, in_=sc, func=EXP, scale=0.125, accum_out=sums
                )
                rec = attn_tmp.tile([128, 1], F32, tag="rec")
                nc.vector.reciprocal(out=rec, in_=sums)
                ean = attn_tmp.tile([128, S], F16, tag="ean")
                nc.vector.tensor_scalar(
                    out=ean, in0=ea, scalar1=rec, scalar2=None, op0=MULT
                )
                nc.tensor.transpose(at_m[:, h, :], ean[:, :128], eye16)
                nc.tensor.transpose(at_t[:, h, :], ean[:, 128:S], eye16)
            atm_sb = attn_tmp.tile([128, 8, 128], F16, tag="atm_sb")
            att_sb = attn_tmp.tile([9, 8, 128], F16, tag="att_sb")
            nc.vector.tensor_copy(atm_sb, at_m)
            nc.scalar.activation(out=att_sb, in_=at_t, func=COPY)
            for h in range(8):
                hp = 64 * (h % 2)
                hkt = h // 2
                nc.tensor.matmul(
                    ctxp[:, h, :],
                    atm_sb[:, h, :],
                    vt_sb[:, b, cq, 64 * h : 64 * h + 64],
                    start=True, stop=False,
                )
                nc.tensor.matmul(
                    ctxp[:, h, :],
                    att_sb[:, h, :],
                    vt_sb[0:9, b, cq + 1, 64 * h : 64 * h + 64],
                    start=False, stop=True,
                )
            ctx_sb = attn_tmp.tile([128, D], F16, tag="ctx_sb")
            nc.scalar.activation(out=ctx_sb, in_=ctxp, func=COPY)
            # transpose ctx [q, hk] -> [hk, q]
            ctp = ps_tr.tile([128, 4, 128], F16, tag="tr")
            for j in range(4):
                nc.tensor.transpose(ctp[:, j, :], ctx_sb[:, 128 * j : 128 * (j + 1)], eye16)
            ctt_sb = attn_tmp.tile([128, 4, 128], F16, tag="ctt_sb")
            nc.vector.tensor_copy(ctt_sb, ctp)
            # out projection + residual (psum slot shared with vT projection)
            op = ps_proj.tile([128, D], F32, tag="pv")
            for j in range(4):
                nc.tensor.matmul(
                    op, ctt_sb[:, j, :], w_sb["o"][:, j, :],
                    start=(j == 0), stop=(j == 3),
                )
            out_sb = attn_tmp.tile([128, D], F32, tag="out_sb")
            nc.scalar.activation(out=out_sb, in_=op, func=COPY)
            # exact fp32 residual on the otherwise-idle GpSimd engine
            nc.gpsimd.tensor_add(out_sb, out_sb, xq32[:, b * NQC + cq, :])
            nc.sync.dma_start(outd[b, 128 * cq : 128 * (cq + 1), :], out_sb)


def _prep_host(inputs):
    """Host-side weight folding and per-core slicing."""
    x = np.asarray(inputs["x"], np.float32)
    gamma = np.asarray(inputs["gamma"], np.float32)
    beta = np.asarray(inputs["beta"], np.float32)
    Wq = np.asarray(inputs["Wq"], np.float32).reshape(D, H * DH)
    Wk = np.asarray(inputs["Wk"], np.float32).reshape(D, H * DH)
    Wv = np.asarray(inputs["Wv"], np.float32).reshape(D, H * DH)
    Wo = np.asarray(inputs["Wo"], np.float32).reshape(H * DH, D)
    bq = np.asarray(inputs["bq"], np.float32).reshape(H * DH)
    bk = np.asarray(inputs["bk"], np.float32).reshape(H * DH)
    bv = np.asarray(inputs["bv"], np.float32).reshape(H * DH)
    bo = np.asarray(inputs["bo"], np.float32).reshape(D)

    Wq2 = gamma[:, None] * Wq
    Wk2 = gamma[:, None] * Wk
    Wv2 = gamma[:, None] * Wv
    cq = bq + beta @ Wq
    ck = bk + beta @ Wk
    cv = bv + beta @ Wv
    if np.any(cq) or np.any(ck):
        raise NotImplementedError("nonzero q/k bias not supported")
    bo_eff = bo + cv @ Wo

    def wtiles(w):  # [D, 512] -> [4, 128, 512] over contraction chunks
        return np.ascontiguousarray(
            w.reshape(4, 128, H * DH).astype(np.float16)
        )

    wq_t = wtiles(Wq2)
    wk_t = wtiles(Wk2)
    wv_t = wtiles(Wv2)
    wo_t = np.ascontiguousarray(Wo.reshape(4, 128, D).astype(np.float16))

    eye16 = np.eye(128, dtype=np.float16)

    in_maps = []
    for c in range(NCORES):
        g0 = TLOC * c - WF
        xs = np.zeros((B, TIN, D), np.float32)
        lo, hi = max(0, g0), min(T, g0 + TIN)
        xs[:, lo - g0 : hi - g0, :] = x[:, lo:hi, :]

        mask = np.full((NQC, 128, S), NEG, np.float16)
        for cqi in range(NQC):
            r = np.arange(128)[:, None]
            sl = np.arange(S)[None, :]
            gj = g0 + 128 * cqi + sl
            valid = (sl - r >= 0) & (sl - r <= WF + WB) & (gj >= 0) & (gj < T)
            mask[cqi][valid] = 0.0

        xq32 = np.ascontiguousarray(
            x[:, TLOC * c : TLOC * (c + 1), :].reshape(B, NQC, 128, D)
        )
        in_maps.append(
            {
                "xs": xs,
                "wq": wq_t, "wk": wk_t, "wv": wv_t, "wo": wo_t,
                "maskd": mask, "eye16": eye16, "xq32": xq32,
            }
        )
    return in_maps, bo_eff


def kernel(**inputs) -> np.ndarray:
    if "nc" not in _CACHE:
        _CACHE["nc"] = _build_program()
    nc = _CACHE["nc"]
    in_maps, bo_eff = _prep_host(inputs)
    res = run_bass_kernel_spmd(nc, in_maps, list(range(NCORES)))
    out = np.empty((B, T, D), np.float32)
    for c in range(NCORES):
        out[:, TLOC * c : TLOC * (c + 1), :] = res.results[c]["out"]
    if np.any(bo_eff):
        out += bo_eff
    return out
